# revision 3
# baseline (speedup 1.0000x reference)
"""MoE feature-dependency model (multilabel) — Trainium2 Bass kernel.

Computation (see reference):
    label_embeds = labels @ label_embed                      # [B, 512]
    combined     = concat([features, label_embeds], -1)      # [B, 2560]
    gating       = sigmoid(combined @ gate_w + gate_b)       # [B, 8]
    expert_out   = einsum('bi,eio->beo', combined, expert_w) + expert_b
    out          = einsum('be,beo->bo', gating, expert_out)  # [B, 2048]

Sharding: the output dimension (2048) is split across the 8 cores — each core
gets a 256-column slice of every expert's weight matrix and computes its slice
of the final gate-weighted sum. No collectives; the host concatenates the
per-core slices along axis 1.

Per-core kernel:
  - features/labels are transposed on the PE (fp32, exact) into combined^T
    laid out K-on-partitions; the PSUM->SBUF copies round to float32r, which
    the expert matmuls consume at full PE rate (1 cycle/row at N=512).
  - experts are processed in pairs: the two 256-wide weight slices sit side by
    side so each matmul streams 512 columns into one PSUM bank.
  - gating is computed transposed ([8, B]) so gate_b and sigmoid ride the
    scalar engine's per-partition bias path, then PE-transposed back to get
    per-row gate scalars for the epilogue.
"""

import numpy as np

import concourse.mybir as mybir
import concourse.tile as tile
from concourse import bacc
from concourse.bass_utils import run_bass_kernel_spmd
from concourse.masks import make_identity

N_CORES = 8
B = 1024          # batch
F = 2048          # feature dim
L = 80            # label dim
ED = 512          # label embed dim
OUT = 2048        # output dim
NE = 8            # experts
IN = F + ED       # 2560 combined dim
NCO = OUT // N_CORES   # 256 output cols per core
KT = IN // 128    # 20 k-tiles of combined
KF = F // 128     # 16 k-tiles from features
MT = B // 128     # 8 m-tiles
NPAIRS = NE // 2  # 4 expert pairs; each pair -> one 512-wide rhs

F32 = mybir.dt.float32
F32R = mybir.dt.float32r
MULT = mybir.AluOpType.mult


def _build():
    nc = bacc.Bacc("TRN2", target_bir_lowering=False, debug=False,
                   num_devices=N_CORES)

    features = nc.dram_tensor("features", [B, F], F32, kind="ExternalInput")
    labels = nc.dram_tensor("labels", [B, L], F32, kind="ExternalInput")
    label_embed = nc.dram_tensor("label_embed", [L, ED], F32, kind="ExternalInput")
    gate_w = nc.dram_tensor("gate_w", [IN, NE], F32, kind="ExternalInput")
    gate_b2 = nc.dram_tensor("gate_b2", [NE, 1], F32, kind="ExternalInput")
    w_pairs = nc.dram_tensor("w_pairs", [NPAIRS, IN, 2 * NCO], F32, kind="ExternalInput")
    b_sl = nc.dram_tensor("b_sl", [NE, NCO], F32, kind="ExternalInput")
    out = nc.dram_tensor("out", [B, NCO], F32, kind="ExternalOutput")

    feat_ap = features.ap()
    gw_view = gate_w.ap().rearrange("(ko p) e -> p ko e", p=128)
    out_view = out.ap().rearrange("(mo p) n -> p mo n", p=128)

    with tile.TileContext(nc) as tc:
        with (
            tc.tile_pool(name="const", bufs=1) as const,
            tc.tile_pool(name="fstage", bufs=2) as fstage,
            tc.tile_pool(name="wstage", bufs=2) as wstage,
            tc.tile_pool(name="wr", bufs=2) as wrpool,
            tc.tile_pool(name="tmp", bufs=4) as tmpp,
            tc.tile_pool(name="psum_mm", bufs=4, space="PSUM") as psum_mm,
            tc.tile_pool(name="psum_misc", bufs=2, space="PSUM") as psum_misc,
            tc.tile_pool(name="psum_g", bufs=1, space="PSUM") as psum_g,
        ):
            ident = const.tile([128, 128], F32)
            make_identity(nc, ident[:])

            # combined^T, K on partitions: k-tiles 0..15 = features^T,
            # 16..19 = label_embeds^T.  float32r via the PSUM->SBUF copies.
            cT = const.tile([128, KT, B], F32R)

            # f32r scratch; regions with disjoint lifetimes overlap.
            misc_r = const.tile([128, 1792], F32R)
            labels_T = misc_r[:L, 0:B]              # early: [80, 1024]
            le_r_all = misc_r[:L, B:B + ED]         # early: [80, 512]
            gT_r = misc_r[:NE, 0:B]                 # late:  [8, 1024]
            b_r = misc_r[:NE, B:B + NCO]            # late:  [8, 256]
            gw_r = misc_r[:, B + ED:B + ED + KT * NE].rearrange(
                "p (k e) -> p k e", e=NE)           # [128, 20, 8]

            # fp32 scratch
            misc32 = const.tile([128, 2048], F32)
            le_st = misc32[:L, 0:ED]                # [80, 512]
            gw_st = misc32[:, ED:ED + KT * NE].rearrange(
                "p (k e) -> p k e", e=NE)           # [128, 20, 8]
            gT = misc32[:NE, 672:672 + B]           # [8, 1024]
            g_sb = misc32[:, 1696:1696 + MT * NE].rearrange(
                "p (m e) -> p m e", e=NE)           # [128, 8, 8]
            b_st = misc32[:NE, 1760:1760 + NCO]     # [8, 256]
            gb_sb = misc32[:NE, 2016:2017]          # [8, 1]

            acc = const.tile([128, MT, NCO], F32)   # output accumulator

            # ---- labels^T (PE transpose, fp32 exact; copy rounds to f32r) --
            for m in range(MT):
                lst = fstage.tile([128, 512], F32, tag="fstage")
                nc.sync.dma_start(lst[:, :L], labels.ap()[m * 128:(m + 1) * 128, :])
                ps = psum_misc.tile([128, 512], F32, tag="ps_misc")
                nc.tensor.transpose(ps[:L, :128], lst[:, :L], ident[:])
                nc.vector.tensor_copy(labels_T[:, m * 128:(m + 1) * 128], ps[:L, :128])

            # ---- label_embeds^T = label_embed^T @ labels^T -> cT[16..19] ---
            nc.sync.dma_start(le_st, label_embed.ap())
            nc.vector.tensor_copy(le_r_all, le_st)
            for j in range(ED // 128):
                for h in range(B // 512):
                    ps = psum_misc.tile([128, 512], F32, tag="ps_misc")
                    nc.tensor.matmul(
                        ps[:],
                        le_r_all[:, j * 128:(j + 1) * 128],
                        labels_T[:, h * 512:(h + 1) * 512],
                        start=True, stop=True,
                    )
                    nc.vector.tensor_copy(
                        cT[:, KF + j, h * 512:(h + 1) * 512], ps[:])

            # ---- features^T -> cT[0..15], k-group-major for early readiness
            for kg in range(4):
                for m in range(MT):
                    fst = fstage.tile([128, 512], F32, tag="fstage")
                    nc.sync.dma_start(
                        fst[:],
                        feat_ap[m * 128:(m + 1) * 128, kg * 512:(kg + 1) * 512])
                    for j in range(4):
                        k = kg * 4 + j
                        ps = psum_misc.tile([128, 512], F32, tag="ps_misc")
                        nc.tensor.transpose(
                            ps[:, :128], fst[:, j * 128:(j + 1) * 128], ident[:])
                        if (m + j) % 2 == 0:
                            nc.vector.tensor_copy(
                                cT[:, k, m * 128:(m + 1) * 128], ps[:, :128])
                        else:
                            nc.scalar.copy(
                                cT[:, k, m * 128:(m + 1) * 128], ps[:, :128])

            # ---- gating^T = sigmoid(gate_w^T @ combined^T + gate_b) --------
            nc.sync.dma_start(gw_st[:], gw_view)
            nc.sync.dma_start(gb_sb, gate_b2.ap())
            nc.vector.tensor_copy(gw_r[:], gw_st[:])
            ps_g = psum_g.tile([NE, B], F32)
            for k in range(KT):
                for h in range(B // 512):
                    nc.tensor.matmul(
                        ps_g[:, h * 512:(h + 1) * 512],
                        gw_r[:, k, :],
                        cT[:, k, h * 512:(h + 1) * 512],
                        start=(k == 0), stop=(k == KT - 1),
                    )
            nc.scalar.activation(gT, ps_g[:], mybir.ActivationFunctionType.Sigmoid,
                                 bias=gb_sb, scale=1.0)
            nc.vector.tensor_copy(gT_r, gT)

            # gate columns per m-tile: [128, m, e] via small PE transposes
            for m in range(MT):
                ps = psum_misc.tile([128, 512], F32, tag="ps_misc")
                nc.tensor.transpose(
                    ps[:, :NE], gT[:, m * 128:(m + 1) * 128], ident[:NE, :NE])
                nc.vector.tensor_copy(g_sb[:, m, :], ps[:, :NE])

            # ---- acc init with gate-weighted expert bias: g_m @ b_sl -------
            nc.sync.dma_start(b_st, b_sl.ap().rearrange("e n -> e n"))
            nc.vector.tensor_copy(b_r, b_st)
            for m in range(MT):
                ps = psum_misc.tile([128, 512], F32, tag="ps_misc")
                nc.tensor.matmul(
                    ps[:, :NCO], gT_r[:, m * 128:(m + 1) * 128], b_r,
                    start=True, stop=True)
                nc.scalar.copy(acc[:, m, :], ps[:, :NCO])

            # ---- expert pairs: MMs + gate-weighted accumulation ------------
            for p in range(NPAIRS):
                wr = wrpool.tile([128, KT, 2 * NCO], F32R, tag="wr")
                for k in range(KT):
                    wst = wstage.tile([128, 2 * NCO], F32, tag="wstage")
                    nc.sync.dma_start(
                        wst[:], w_pairs.ap()[p, k * 128:(k + 1) * 128, :])
                    nc.gpsimd.tensor_copy(wr[:, k, :], wst[:])
                for m in range(MT):
                    ps = psum_mm.tile([128, 2 * NCO], F32, tag="ps_mm")
                    for k in range(KT):
                        nc.tensor.matmul(
                            ps[:],
                            cT[:, k, m * 128:(m + 1) * 128],
                            wr[:, k, :],
                            start=(k == 0), stop=(k == KT - 1),
                        )
                    tmp = tmpp.tile([128, 2 * NCO], F32, tag="tmp")
                    nc.scalar.activation(
                        tmp[:, :NCO], ps[:, :NCO],
                        mybir.ActivationFunctionType.Copy,
                        scale=g_sb[:, m, 2 * p:2 * p + 1])
                    nc.scalar.activation(
                        tmp[:, NCO:], ps[:, NCO:],
                        mybir.ActivationFunctionType.Copy,
                        scale=g_sb[:, m, 2 * p + 1:2 * p + 2])
                    nc.vector.tensor_add(acc[:, m, :], acc[:, m, :], tmp[:, :NCO])
                    nc.vector.tensor_add(acc[:, m, :], acc[:, m, :], tmp[:, NCO:])

            # ---- store ------------------------------------------------------
            for m in range(MT):
                ost = wstage.tile([128, 2 * NCO], F32, tag="wstage")
                nc.vector.tensor_copy(ost[:, :NCO], acc[:, m, :])
                nc.sync.dma_start(out_view[:, m], ost[:, :NCO])

    nc.compile()
    return nc


_NC_CACHE = None


def kernel(features, labels, label_embed, gate_w, gate_b, expert_w, expert_b):
    global _NC_CACHE
    if _NC_CACHE is None:
        _NC_CACHE = _build()
    nc = _NC_CACHE

    features = np.ascontiguousarray(np.asarray(features, dtype=np.float32))
    labels = np.ascontiguousarray(np.asarray(labels, dtype=np.float32))
    label_embed = np.ascontiguousarray(np.asarray(label_embed, dtype=np.float32))
    gate_w = np.ascontiguousarray(np.asarray(gate_w, dtype=np.float32))
    gate_b = np.asarray(gate_b, dtype=np.float32).reshape(NE, 1)
    expert_w = np.asarray(expert_w, dtype=np.float32)
    expert_b = np.asarray(expert_b, dtype=np.float32)

    in_maps = []
    for c in range(N_CORES):
        sl = slice(c * NCO, (c + 1) * NCO)
        w = expert_w[:, :, sl]  # [8, 2560, 256]
        w_pairs = np.ascontiguousarray(
            np.stack([np.concatenate([w[2 * p], w[2 * p + 1]], axis=1)
                      for p in range(NPAIRS)]))
        in_maps.append({
            "features": features,
            "labels": labels,
            "label_embed": label_embed,
            "gate_w": gate_w,
            "gate_b2": gate_b,
            "w_pairs": w_pairs,
            "b_sl": np.ascontiguousarray(expert_b[:, sl]),
        })

    res = run_bass_kernel_spmd(nc, in_maps, core_ids=list(range(N_CORES)))
    return np.concatenate([res.results[c]["out"] for c in range(N_CORES)], axis=1)


# revision 4
# speedup vs baseline: 1.0261x; 1.0261x over previous
"""MoE feature-dependency model (multilabel) — Trainium2 Bass kernel.

Computation (see reference):
    label_embeds = labels @ label_embed                      # [B, 512]
    combined     = concat([features, label_embeds], -1)      # [B, 2560]
    gating       = sigmoid(combined @ gate_w + gate_b)       # [B, 8]
    expert_out   = einsum('bi,eio->beo', combined, expert_w) + expert_b
    out          = einsum('be,beo->bo', gating, expert_out)  # [B, 2048]

Sharding: the output dimension (2048) is split across the 8 cores — each core
gets a 256-column slice of every expert's weight stack and computes its slice
of the final gate-weighted sum. No collectives; the host concatenates the
per-core slices along axis 1.

Per-core kernel:
  - features^T / labels^T are prepared host-side (pure layout prep, like the
    expert-pair packing); on device they are DMA'd and rounded to float32r,
    which the PE consumes at full rate (1 cycle/row at N=512; plain fp32
    matmul runs at 1/4 rate).  label_embeds^T is computed on the PE.
  - experts are processed in pairs: the two 256-wide weight slices sit side by
    side so each matmul streams 512 columns into exactly one PSUM bank.
  - gating is computed transposed ([8, B]) with its matmuls interleaved into
    the combined^T k-tile arrival, so gate_b + sigmoid ride the scalar
    engine's per-partition bias path and gating is ready when the expert
    stream starts.
"""

import numpy as np

import concourse.mybir as mybir
import concourse.tile as tile
from concourse import bacc
from concourse.bass_utils import run_bass_kernel_spmd
from concourse.masks import make_identity

N_CORES = 8
B = 1024          # batch
F = 2048          # feature dim
L = 80            # label dim
ED = 512          # label embed dim
OUT = 2048        # output dim
NE = 8            # experts
IN = F + ED       # 2560 combined dim
NCO = OUT // N_CORES   # 256 output cols per core
KT = IN // 128    # 20 k-tiles of combined
KF = F // 128     # 16 k-tiles from features
MT = B // 128     # 8 m-tiles
NPAIRS = NE // 2  # 4 expert pairs; each pair -> one 512-wide rhs

F32 = mybir.dt.float32
F32R = mybir.dt.float32r


def _build():
    nc = bacc.Bacc("TRN2", target_bir_lowering=False, debug=False,
                   num_devices=N_CORES)

    feat_t = nc.dram_tensor("feat_t", [F, B], F32, kind="ExternalInput")
    labels_t = nc.dram_tensor("labels_t", [L, B], F32, kind="ExternalInput")
    label_embed = nc.dram_tensor("label_embed", [L, ED], F32, kind="ExternalInput")
    gate_w = nc.dram_tensor("gate_w", [IN, NE], F32, kind="ExternalInput")
    gate_b2 = nc.dram_tensor("gate_b2", [NE, 1], F32, kind="ExternalInput")
    w_pairs = nc.dram_tensor("w_pairs", [NPAIRS, IN, 2 * NCO], F32, kind="ExternalInput")
    b_sl = nc.dram_tensor("b_sl", [NE, NCO], F32, kind="ExternalInput")
    out = nc.dram_tensor("out", [B, NCO], F32, kind="ExternalOutput")

    ft_view = feat_t.ap().rearrange("(ko p) m -> p ko m", p=128)
    gw_view = gate_w.ap().rearrange("(ko p) e -> p ko e", p=128)
    out_view = out.ap().rearrange("(mo p) n -> p mo n", p=128)

    with tile.TileContext(nc) as tc:
        with (
            tc.tile_pool(name="const", bufs=1) as const,
            tc.tile_pool(name="fstage", bufs=3) as fstage,
            tc.tile_pool(name="wstage", bufs=2) as wstage,
            tc.tile_pool(name="wr", bufs=2) as wrpool,
            tc.tile_pool(name="tmp", bufs=4) as tmpp,
            tc.tile_pool(name="psum_mm", bufs=4, space="PSUM") as psum_mm,
            tc.tile_pool(name="psum_misc", bufs=2, space="PSUM") as psum_misc,
            tc.tile_pool(name="psum_g", bufs=1, space="PSUM") as psum_g,
        ):
            ident = const.tile([128, 128], F32)
            make_identity(nc, ident[:])

            # combined^T, K on partitions: k-tiles 0..15 = features^T,
            # 16..19 = label_embeds^T.  float32r via compute-op copies.
            cT = const.tile([128, KT, B], F32R)

            # f32r scratch; regions with disjoint lifetimes overlap.
            misc_r = const.tile([128, 1792], F32R)
            labels_T = misc_r[:L, 0:B]              # early: [80, 1024]
            le_r_all = misc_r[:L, B:B + ED]         # early: [80, 512]
            gT_r = misc_r[:NE, 0:B]                 # late:  [8, 1024]
            b_r = misc_r[:NE, B:B + NCO]            # late:  [8, 256]
            gw_r = misc_r[:, B + ED:B + ED + KT * NE].rearrange(
                "p (k e) -> p k e", e=NE)           # [128, 20, 8]

            # fp32 scratch
            misc32 = const.tile([128, 2048], F32)
            le_st = misc32[:L, 0:ED]                # [80, 512]
            gw_st = misc32[:, ED:ED + KT * NE].rearrange(
                "p (k e) -> p k e", e=NE)           # [128, 20, 8]
            gT = misc32[:NE, 672:672 + B]           # [8, 1024]
            g_sb = misc32[:, 1696:1696 + MT * NE].rearrange(
                "p (m e) -> p m e", e=NE)           # [128, 8, 8]
            b_st = misc32[:NE, 1760:1760 + NCO]     # [8, 256]
            gb_sb = misc32[:NE, 2016:2017]          # [8, 1]

            acc = const.tile([128, MT, NCO], F32)   # output accumulator

            # gating matmul emitter — interleaved with cT k-tile arrival.
            # PE executes in emission order, so emit k=16..19 (labels part,
            # ready first) with start on k==16, then k=0..15, stop on k==15.
            ps_g = psum_g.tile([NE, B], F32)

            def gate_mm(k):
                for h in range(B // 512):
                    nc.tensor.matmul(
                        ps_g[:, h * 512:(h + 1) * 512],
                        gw_r[:, k, :],
                        cT[:, k, h * 512:(h + 1) * 512],
                        start=(k == KF), stop=(k == KF - 1),
                    )

            # ---- gate_w / labels^T / label_embed staging -------------------
            nc.sync.dma_start(gw_st[:], gw_view)
            nc.sync.dma_start(gb_sb, gate_b2.ap())
            nc.vector.tensor_copy(gw_r[:], gw_st[:])

            lst = fstage.tile([128, B], F32, tag="fstage")
            nc.sync.dma_start(lst[:L, :], labels_t.ap())
            nc.vector.tensor_copy(labels_T, lst[:L, :])
            nc.sync.dma_start(le_st, label_embed.ap())
            nc.vector.tensor_copy(le_r_all, le_st)

            # ---- label_embeds^T = label_embed^T @ labels^T -> cT[16..19] ---
            for j in range(ED // 128):
                for h in range(B // 512):
                    ps = psum_misc.tile([128, 512], F32, tag="ps_misc")
                    nc.tensor.matmul(
                        ps[:],
                        le_r_all[:, j * 128:(j + 1) * 128],
                        labels_T[:, h * 512:(h + 1) * 512],
                        start=True, stop=True,
                    )
                    nc.vector.tensor_copy(
                        cT[:, KF + j, h * 512:(h + 1) * 512], ps[:])
            for k in range(KF, KT):
                gate_mm(k)

            # ---- features^T -> cT[0..15] (DMA + rounding copy) -------------
            for k in range(KF):
                fst = fstage.tile([128, B], F32, tag="fstage")
                nc.sync.dma_start(fst[:], ft_view[:, k])
                if k % 2 == 0:
                    nc.vector.tensor_copy(cT[:, k, :], fst[:])
                else:
                    nc.scalar.copy(cT[:, k, :], fst[:])
                gate_mm(k)

            # ---- sigmoid(+gate_b), gate columns per m-tile -----------------
            nc.scalar.activation(gT, ps_g[:], mybir.ActivationFunctionType.Sigmoid,
                                 bias=gb_sb, scale=1.0)
            nc.vector.tensor_copy(gT_r, gT)
            for m in range(MT):
                ps = psum_misc.tile([128, 512], F32, tag="ps_misc")
                nc.tensor.transpose(
                    ps[:, :NE], gT[:, m * 128:(m + 1) * 128], ident[:NE, :NE])
                nc.vector.tensor_copy(g_sb[:, m, :], ps[:, :NE])

            # ---- acc init with gate-weighted expert bias: g_m @ b_sl -------
            nc.sync.dma_start(b_st, b_sl.ap())
            nc.vector.tensor_copy(b_r, b_st)
            for m in range(MT):
                ps = psum_misc.tile([128, 512], F32, tag="ps_misc")
                nc.tensor.matmul(
                    ps[:, :NCO], gT_r[:, m * 128:(m + 1) * 128], b_r,
                    start=True, stop=True)
                nc.scalar.copy(acc[:, m, :], ps[:, :NCO])

            # ---- expert pairs: MMs + gate-weighted accumulation ------------
            for p in range(NPAIRS):
                wr = wrpool.tile([128, KT, 2 * NCO], F32R, tag="wr")
                for k in range(KT):
                    wst = wstage.tile([128, 2 * NCO], F32, tag="wstage")
                    nc.sync.dma_start(
                        wst[:], w_pairs.ap()[p, k * 128:(k + 1) * 128, :])
                    nc.gpsimd.tensor_copy(wr[:, k, :], wst[:])
                for m in range(MT):
                    ps = psum_mm.tile([128, 2 * NCO], F32, tag="ps_mm")
                    for k in range(KT):
                        nc.tensor.matmul(
                            ps[:],
                            cT[:, k, m * 128:(m + 1) * 128],
                            wr[:, k, :],
                            start=(k == 0), stop=(k == KT - 1),
                        )
                    tmp = tmpp.tile([128, 2 * NCO], F32, tag="tmp")
                    nc.scalar.activation(
                        tmp[:, :NCO], ps[:, :NCO],
                        mybir.ActivationFunctionType.Copy,
                        scale=g_sb[:, m, 2 * p:2 * p + 1])
                    nc.scalar.activation(
                        tmp[:, NCO:], ps[:, NCO:],
                        mybir.ActivationFunctionType.Copy,
                        scale=g_sb[:, m, 2 * p + 1:2 * p + 2])
                    nc.vector.tensor_add(acc[:, m, :], acc[:, m, :], tmp[:, :NCO])
                    nc.vector.tensor_add(acc[:, m, :], acc[:, m, :], tmp[:, NCO:])

            # ---- store -----------------------------------------------------
            for m in range(MT):
                ost = wstage.tile([128, 2 * NCO], F32, tag="wstage")
                nc.vector.tensor_copy(ost[:, :NCO], acc[:, m, :])
                nc.sync.dma_start(out_view[:, m], ost[:, :NCO])

    nc.compile()
    return nc


_NC_CACHE = None


def make_in_maps(features, labels, label_embed, gate_w, gate_b, expert_w, expert_b):
    features = np.asarray(features, dtype=np.float32)
    labels = np.asarray(labels, dtype=np.float32)
    feat_t = np.ascontiguousarray(features.T)
    labels_t = np.ascontiguousarray(labels.T)
    label_embed = np.ascontiguousarray(np.asarray(label_embed, dtype=np.float32))
    gate_w = np.ascontiguousarray(np.asarray(gate_w, dtype=np.float32))
    gate_b = np.asarray(gate_b, dtype=np.float32).reshape(NE, 1)
    expert_w = np.asarray(expert_w, dtype=np.float32)
    expert_b = np.asarray(expert_b, dtype=np.float32)

    # [8, 2560, 2048] -> per-core [4 pairs, 2560, 512]: pair 2p/2p+1 slices
    # side by side.
    ew = expert_w.reshape(NPAIRS, 2, IN, N_CORES, NCO)
    in_maps = []
    for c in range(N_CORES):
        sl = slice(c * NCO, (c + 1) * NCO)
        wp = np.concatenate([ew[:, 0, :, c, :], ew[:, 1, :, c, :]], axis=2)
        in_maps.append({
            "feat_t": feat_t,
            "labels_t": labels_t,
            "label_embed": label_embed,
            "gate_w": gate_w,
            "gate_b2": gate_b,
            "w_pairs": np.ascontiguousarray(wp),
            "b_sl": np.ascontiguousarray(expert_b[:, sl]),
        })
    return in_maps


def kernel(features, labels, label_embed, gate_w, gate_b, expert_w, expert_b):
    global _NC_CACHE
    if _NC_CACHE is None:
        _NC_CACHE = _build()
    nc = _NC_CACHE
    in_maps = make_in_maps(features, labels, label_embed, gate_w, gate_b,
                           expert_w, expert_b)
    res = run_bass_kernel_spmd(nc, in_maps, core_ids=list(range(N_CORES)))
    return np.concatenate([res.results[c]["out"] for c in range(N_CORES)], axis=1)


# revision 18
# speedup vs baseline: 1.3210x; 1.2875x over previous
"""MoE feature-dependency model (multilabel) — Trainium2 Bass kernel.

Computation (see reference):
    label_embeds = labels @ label_embed                      # [B, 512]
    combined     = concat([features, label_embeds], -1)      # [B, 2560]
    gating       = sigmoid(combined @ gate_w + gate_b)       # [B, 8]
    expert_out   = einsum('bi,eio->beo', combined, expert_w) + expert_b
    out          = einsum('be,beo->bo', gating, expert_out)  # [B, 2048]

Sharding: the output dimension (2048) is split across the 8 cores — each core
gets a 256-column slice of every expert's weight stack and computes its slice
of the final gate-weighted sum. No collectives; the host concatenates the
per-core slices along axis 1.

Per-core kernel:
  - features^T / labels^T are prepared host-side (pure layout prep, like the
    expert-pair packing); on device they are DMA'd and rounded to float32r,
    which the PE consumes at full rate (1 cycle/row at N=512; plain fp32
    matmul runs at 1/4 rate).  label_embeds^T is computed on the PE.
  - experts are processed in pairs: the two 256-wide weight slices sit side by
    side so each matmul streams 512 columns into exactly one PSUM bank.
  - gating is computed transposed ([8, B]) with its matmuls interleaved into
    the combined^T k-tile arrival, so gate_b + sigmoid ride the scalar
    engine's per-partition bias path and gating is ready when the expert
    stream starts.
"""

from contextlib import ExitStack

import numpy as np

import concourse.mybir as mybir
import concourse.tile as tile
from concourse import bacc
from concourse.bass_utils import run_bass_kernel_spmd
from concourse.masks import make_identity

N_CORES = 8
B = 1024          # batch
F = 2048          # feature dim
L = 80            # label dim
ED = 512          # label embed dim
OUT = 2048        # output dim
NE = 8            # experts
IN = F + ED       # 2560 combined dim
NCO = OUT // N_CORES   # 256 output cols per core
KT = IN // 128    # 20 k-tiles of combined
KF = F // 128     # 16 k-tiles from features
MT = B // 128     # 8 m-tiles
NPAIRS = NE // 2  # 4 expert pairs; each pair -> one 512-wide rhs

F32 = mybir.dt.float32
F32R = mybir.dt.float32r


def _build():
    nc = bacc.Bacc("TRN2", target_bir_lowering=False, debug=False,
                   num_devices=N_CORES)

    feat_t = nc.dram_tensor("feat_t", [F, B], F32, kind="ExternalInput")
    labels_t = nc.dram_tensor("labels_t", [L, B], F32, kind="ExternalInput")
    label_embed = nc.dram_tensor("label_embed", [L, ED], F32, kind="ExternalInput")
    gate_w = nc.dram_tensor("gate_w", [IN, NE], F32, kind="ExternalInput")
    gate_b2 = nc.dram_tensor("gate_b2", [NE, 1], F32, kind="ExternalInput")
    w_pairs = nc.dram_tensor("w_pairs", [NPAIRS, IN, 2 * NCO], F32, kind="ExternalInput")
    b_sl = nc.dram_tensor("b_sl", [NE, NCO], F32, kind="ExternalInput")
    out = nc.dram_tensor("out", [B, NCO], F32, kind="ExternalOutput")

    ft_view = feat_t.ap().rearrange("(ko p) m -> p ko m", p=128)
    gw_view = gate_w.ap().rearrange("(ko p) e -> p ko e", p=128)
    out_view = out.ap().rearrange("(mo p) n -> p mo n", p=128)

    with tile.TileContext(nc) as tc:
        with (
            tc.tile_pool(name="const", bufs=1) as const,
            tc.tile_pool(name="fstage", bufs=3) as fstage,
            tc.tile_pool(name="wstage", bufs=4) as wstage,
            tc.tile_pool(name="wr", bufs=2) as wrpool,
            tc.tile_pool(name="tmp", bufs=2) as tmpp,
            tc.tile_pool(name="psum_mm", bufs=6, space="PSUM") as psum_mm,
        ):
            ident = const.tile([128, 128], F32)
            make_identity(nc, ident[:])

            # combined^T, K on partitions: k-tiles 0..15 = features^T,
            # 16..19 = label_embeds^T.  float32r via compute-op copies.
            cT = const.tile([128, KT, B], F32R)

            # f32r scratch; regions with disjoint lifetimes overlap.
            misc_r = const.tile([128, 1792], F32R)
            labels_T = misc_r[:L, 0:B]              # early: [80, 1024]
            le_r_all = misc_r[:L, B:B + ED]         # early: [80, 512]
            gT_r = misc_r[:NE, 0:B]                 # late:  [8, 1024]
            b_r = misc_r[:NE, B:B + NCO]            # late:  [8, 256]
            gw_r = misc_r[:, B + ED:B + ED + KT * NE].rearrange(
                "p (k e) -> p k e", e=NE)           # [128, 20, 8]

            # fp32 scratch
            misc32 = const.tile([128, 2048], F32)
            le_st = misc32[:L, 0:ED]                # [80, 512]
            gw_st = misc32[:, ED:ED + KT * NE].rearrange(
                "p (k e) -> p k e", e=NE)           # [128, 20, 8]
            gT = misc32[:NE, 672:672 + B]           # [8, 1024]
            g_sb = misc32[:, 1696:1696 + MT * NE].rearrange(
                "p (m e) -> p m e", e=NE)           # [128, 8, 8]
            b_st = misc32[:NE, 1760:1760 + NCO]     # [8, 256]
            gb_sb = misc32[:NE, 2016:2017]          # [8, 1]

            acc = const.tile([128, MT, NCO], F32)   # output accumulator

            # gating matmul emitter — interleaved with cT k-tile arrival.
            # PE executes in emission order, so emit k=16..19 (labels part,
            # ready first) with start on k==16, then k=0..15, stop on k==15.
            # The gating psum pool is scoped to the head: its 2 banks are
            # released after the sigmoid so the post-head misc pool can use
            # them (6 mm + 2 = 8 banks at any time).
            g_stack = ExitStack()
            psum_g = g_stack.enter_context(
                tc.tile_pool(name="psum_g", bufs=1, space="PSUM"))
            ps_g = psum_g.tile([NE, B], F32, name="ps_g")

            def gate_mm(k):
                for h in range(B // 512):
                    nc.tensor.matmul(
                        ps_g[:, h * 512:(h + 1) * 512],
                        gw_r[:, k, :],
                        cT[:, k, h * 512:(h + 1) * 512],
                        start=(k == KF), stop=(k == KF - 1),
                    )

            # ---- labels^T / label_embed / gate_w staging -------------------
            lst = fstage.tile([128, B], F32, tag="fstage")
            nc.sync.dma_start(lst[:L, :], labels_t.ap())
            nc.vector.tensor_copy(labels_T, lst[:L, :])
            nc.sync.dma_start(le_st, label_embed.ap())
            nc.vector.tensor_copy(le_r_all, le_st)

            nc.sync.dma_start(gw_st[:], gw_view)
            nc.sync.dma_start(gb_sb, gate_b2.ap())
            nc.vector.tensor_copy(gw_r[:], gw_st[:])

            # ---- expert pair 0 weights, loaded during the head -------------
            def stage_w(wr, p, k):
                wst = wstage.tile([128, 2 * NCO], F32, tag="wstage")
                nc.sync.dma_start(
                    wst[:], w_pairs.ap()[p, k * 128:(k + 1) * 128, :])
                nc.gpsimd.tensor_copy(wr[:, k, :], wst[:])

            def load_w(p):
                wr = wrpool.tile([128, KT, 2 * NCO], F32R, tag="wr")
                for k in range(KT):
                    stage_w(wr, p, k)
                return wr

            # pair 0 consumes k=16..19 first (label part of cT lands first);
            # its k=0..15 weight tiles are interleaved with the feature DMAs
            # below so neither stream monopolizes the DMA queues.
            wr0 = wrpool.tile([128, KT, 2 * NCO], F32R, tag="wr", name="wr0")
            for k in range(KF, KT):
                stage_w(wr0, 0, k)

            # ---- label_embeds^T = label_embed^T @ labels^T -> cT[16..19] ---
            # (transient tiles from the mm pool, released before ps0 starts)
            for j in range(ED // 128):
                for h in range(B // 512):
                    ps = psum_mm.tile([128, 2 * NCO], F32, tag="ps_mm")
                    nc.tensor.matmul(
                        ps[:],
                        le_r_all[:, j * 128:(j + 1) * 128],
                        labels_T[:, h * 512:(h + 1) * 512],
                        start=True, stop=True,
                    )
                    nc.vector.tensor_copy(
                        cT[:, KF + j, h * 512:(h + 1) * 512], ps[:])

            # pair-0 m0..5 accumulate k-outer, interleaved with cT arrival so
            # the PE has work while the feature DMA stream lands.
            KO_M = 6
            ps0 = [psum_mm.tile([128, 2 * NCO], F32, tag="ps_mm",
                                name=f"ps0_{m}")
                   for m in range(KO_M)]

            def p0_mm(k):
                for m in range(KO_M):
                    nc.tensor.matmul(
                        ps0[m],
                        cT[:, k, m * 128:(m + 1) * 128],
                        wr0[:, k, :],
                        start=(k == KF), stop=(k == KF - 1),
                    )

            for k in range(KF, KT):
                gate_mm(k)
                p0_mm(k)

            # ---- features^T -> cT[0..15] (DMA + rounding copy),
            # interleaved with pair-0 weight staging ------------------------
            for k in range(KF):
                fst = fstage.tile([128, B], F32, tag="fstage")
                nc.sync.dma_start(fst[:], ft_view[:, k])
                stage_w(wr0, 0, k)
                if k % 2 == 0:
                    nc.vector.tensor_copy(cT[:, k, :], fst[:])
                else:
                    nc.scalar.copy(cT[:, k, :], fst[:])
                gate_mm(k)
                p0_mm(k)

            # ---- sigmoid(+gate_b), gate columns per m-tile -----------------
            nc.scalar.activation(gT, ps_g[:], mybir.ActivationFunctionType.Sigmoid,
                                 bias=gb_sb, scale=1.0)
            g_stack.close()  # release the gating psum banks
            psum_misc = g_stack.enter_context(
                tc.tile_pool(name="psum_misc", bufs=2, space="PSUM"))
            nc.vector.tensor_copy(gT_r, gT)
            for m in range(MT):
                ps = psum_misc.tile([128, 512], F32, tag="ps_misc")
                nc.tensor.transpose(
                    ps[:, :NE], gT[:, m * 128:(m + 1) * 128], ident[:NE, :NE])
                nc.vector.tensor_copy(g_sb[:, m, :], ps[:, :NE])

            # ---- acc init with gate-weighted expert bias: g_m @ b_sl -------
            nc.sync.dma_start(b_st, b_sl.ap())
            nc.vector.tensor_copy(b_r, b_st)
            for m in range(MT):
                ps = psum_misc.tile([128, 512], F32, tag="ps_misc")
                nc.tensor.matmul(
                    ps[:, :NCO], gT_r[:, m * 128:(m + 1) * 128], b_r,
                    start=True, stop=True)
                nc.scalar.copy(acc[:, m, :], ps[:, :NCO])

            # ---- expert pairs: MMs + gate-weighted accumulation ------------
            def epilogue(p, m, ps):
                tmp = tmpp.tile([128, 2 * NCO], F32, tag="tmp")
                nc.scalar.activation(
                    tmp[:, :NCO], ps[:, :NCO],
                    mybir.ActivationFunctionType.Copy,
                    scale=g_sb[:, m, 2 * p:2 * p + 1])
                nc.scalar.activation(
                    tmp[:, NCO:], ps[:, NCO:],
                    mybir.ActivationFunctionType.Copy,
                    scale=g_sb[:, m, 2 * p + 1:2 * p + 2])
                nc.vector.tensor_add(acc[:, m, :], acc[:, m, :], tmp[:, :NCO])
                nc.vector.tensor_add(acc[:, m, :], acc[:, m, :], tmp[:, NCO:])

            for m in range(KO_M):
                epilogue(0, m, ps0[m])

            for p in range(NPAIRS):
                wr = wr0 if p == 0 else load_w(p)
                for m in range(KO_M if p == 0 else 0, MT):
                    ps = psum_mm.tile([128, 2 * NCO], F32, tag="ps_mm")
                    for k in range(KT):
                        nc.tensor.matmul(
                            ps[:],
                            cT[:, k, m * 128:(m + 1) * 128],
                            wr[:, k, :],
                            start=(k == 0), stop=(k == KT - 1),
                        )
                    epilogue(p, m, ps)

            # ---- store -----------------------------------------------------
            for m in range(MT):
                ost = wstage.tile([128, 2 * NCO], F32, tag="wstage")
                nc.vector.tensor_copy(ost[:, :NCO], acc[:, m, :])
                nc.sync.dma_start(out_view[:, m], ost[:, :NCO])

            g_stack.close()

    nc.compile()
    return nc


_NC_CACHE = None


def make_in_maps(features, labels, label_embed, gate_w, gate_b, expert_w, expert_b):
    features = np.asarray(features, dtype=np.float32)
    labels = np.asarray(labels, dtype=np.float32)
    feat_t = np.ascontiguousarray(features.T)
    labels_t = np.ascontiguousarray(labels.T)
    label_embed = np.ascontiguousarray(np.asarray(label_embed, dtype=np.float32))
    gate_w = np.ascontiguousarray(np.asarray(gate_w, dtype=np.float32))
    gate_b = np.asarray(gate_b, dtype=np.float32).reshape(NE, 1)
    expert_w = np.asarray(expert_w, dtype=np.float32)
    expert_b = np.asarray(expert_b, dtype=np.float32)

    # [8, 2560, 2048] -> per-core [4 pairs, 2560, 512]: pair 2p/2p+1 slices
    # side by side.
    ew = expert_w.reshape(NPAIRS, 2, IN, N_CORES, NCO)
    in_maps = []
    for c in range(N_CORES):
        sl = slice(c * NCO, (c + 1) * NCO)
        wp = np.concatenate([ew[:, 0, :, c, :], ew[:, 1, :, c, :]], axis=2)
        in_maps.append({
            "feat_t": feat_t,
            "labels_t": labels_t,
            "label_embed": label_embed,
            "gate_w": gate_w,
            "gate_b2": gate_b,
            "w_pairs": np.ascontiguousarray(wp),
            "b_sl": np.ascontiguousarray(expert_b[:, sl]),
        })
    return in_maps


def kernel(features, labels, label_embed, gate_w, gate_b, expert_w, expert_b):
    global _NC_CACHE
    if _NC_CACHE is None:
        _NC_CACHE = _build()
    nc = _NC_CACHE
    in_maps = make_in_maps(features, labels, label_embed, gate_w, gate_b,
                           expert_w, expert_b)
    res = run_bass_kernel_spmd(nc, in_maps, core_ids=list(range(N_CORES)))
    return np.concatenate([res.results[c]["out"] for c in range(N_CORES)], axis=1)
